# revision 1
# baseline (speedup 1.0000x reference)
"""Trainium2 Bass kernel for the IWE (image-warped-events) problem.

Full inputs in, full outputs out. Data-parallel over (batch, half) across 8
NeuronCores; each core computes a partial IWE grid over its events plus the
avg_flow channels; host sums the two partial IWEs per batch.

Per-core pipeline, per 500-event chunk (4 blocks x 125 events on partitions):
  - one-hot(y) via rank-2 matmul (y_e - y) into PSUM + is_equal(0)
  - flow gather: onehotY @ [fy|fx] rows; x-select via product with onehot(x)
    and a blocked free-dim reduction
  - warped coords; bilinear "hat" weight rows hat(t)=relu(1-|t|) on iota grids
    (corner weights + OOB masking in one shot)
  - scatter-add via PSUM-accumulated matmuls: SUM = hatY^T@hatX,
    DIFF = (hatY*sgn)^T@hatX;  pos=(SUM+DIFF)/2, neg=(SUM-DIFF)/2
"""
import numpy as np

H, W = 128, 128
NCORES = 8
CHUNK = 500                            # events per chunk
PBLK = 125                             # partition rows used for events
NBLK = 4                               # blocks per chunk (PBLK*NBLK = CHUNK)

_COMPILED = {}


def _build(nchunks, use_hw_loop=True, unroll=4, passes=1):
    import concourse.bass as bass
    import concourse.bacc as bacc
    import concourse.mybir as mybir
    from concourse.tile import TileContext

    fp32 = mybir.dt.float32
    bf16 = mybir.dt.bfloat16
    int32 = mybir.dt.int32
    Alu = mybir.AluOpType
    E = nchunks * CHUNK

    nc = bacc.Bacc("TRN2", target_bir_lowering=False, debug=False,
                   num_devices=NCORES)

    ev = nc.dram_tensor("ev", [E, 4], fp32, kind="ExternalInput").ap()
    pol = nc.dram_tensor("pol", [E, 2], fp32, kind="ExternalInput").ap()
    flow = nc.dram_tensor("flow", [2, H, W], fp32, kind="ExternalInput").ap()
    emask = nc.dram_tensor("emask", [H, W], fp32, kind="ExternalInput").ap()
    out = nc.dram_tensor("out", [4, H, W], fp32, kind="ExternalOutput").ap()

    # event id e = c*CHUNK + p*NBLK + n  (chunk c, partition p, block n)
    ev_v = ev.rearrange("(c p n) f -> p c (n f)", c=nchunks, p=PBLK, n=NBLK)
    evy_v = ev.rearrange("(c p n) f -> n c p f", c=nchunks, p=PBLK, n=NBLK)
    pol_v = pol.rearrange("(c p n) f -> p c (n f)", c=nchunks, p=PBLK, n=NBLK)

    with TileContext(nc) as tc:
        with tc.tile_pool(name="const", bufs=1) as cpool, \
             tc.tile_pool(name="work", bufs=2) as wpool, \
             tc.tile_pool(name="ppool", bufs=1, space="PSUM") as ppool:

            # ---------------- constants ----------------
            iotai = cpool.tile([128, 128], int32)
            nc.gpsimd.iota(iotai[:], pattern=[[1, 128]], base=0, channel_multiplier=0)
            iota32 = cpool.tile([128, 128], fp32)
            nc.vector.tensor_copy(out=iota32[:], in_=iotai[:])
            # dy matmul weights: lhsT_j [16,128]: row j = ones, row 4 = -iota
            pidi = cpool.tile([16, 1], int32)
            nc.gpsimd.iota(pidi[:], pattern=[[1, 1]], base=0, channel_multiplier=1)
            pidf = cpool.tile([16, 1], fp32)
            nc.vector.tensor_copy(out=pidf[:], in_=pidi[:])
            neq4 = cpool.tile([16, 1], fp32)
            nc.vector.tensor_scalar(out=neq4[:], in0=pidf[:], scalar1=4.0,
                                    scalar2=-1.0, op0=Alu.is_equal, op1=Alu.mult)
            dylhs = []
            for j in range(NBLK):
                eqj = cpool.tile([16, 1], fp32, tag=f"eq{j}")
                nc.vector.tensor_scalar(out=eqj[:], in0=pidf[:], scalar1=float(j),
                                        scalar2=None, op0=Alu.is_equal)
                lt = cpool.tile([16, 128], bf16, tag=f"dylhs{j}")
                nc.vector.scalar_tensor_tensor(
                    out=lt[:], in0=iota32[0:16, :], scalar=neq4[:],
                    in1=eqj[:].broadcast_to([16, 128]), op0=Alu.mult, op1=Alu.add)
                dylhs.append(lt)
            # y-row staging: rows 0..3 get per-chunk y rows, rows 4..15 stay 1.0
            yb16 = cpool.tile([16, 128], bf16)
            nc.vector.memset(yb16[:], 1.0)

            # flow rows bf16: [y, 0:128]=flow[1] (fy), [y, 128:256]=flow[0] (fx)
            flow32 = cpool.tile([128, 256], fp32)
            nc.sync.dma_start(out=flow32[:, 0:128], in_=flow[1])
            nc.sync.dma_start(out=flow32[:, 128:256], in_=flow[0])
            flowcat = cpool.tile([128, 256], bf16)
            nc.vector.tensor_copy(out=flowcat[:], in_=flow32[:])

            # ---------------- psum tiles ----------------
            p_dy = ppool.tile([128, CHUNK], fp32, tag="dy")        # [y, e]
            p_t1 = ppool.tile([PBLK, NBLK * 256], fp32, tag="t1")  # [e, blk*(fy|fx)]
            p_gs = ppool.tile([128, 128], fp32, tag="gsum")
            p_gd = ppool.tile([128, 128], fp32, tag="gdiff")
            nc.vector.memset(p_gs[:], 0.0)
            nc.vector.memset(p_gd[:], 0.0)

            def body(i):
                evt = wpool.tile([PBLK, NBLK * 4], fp32, tag="evt")
                polt = wpool.tile([PBLK, NBLK * 2], fp32, tag="polt")
                nc.sync.dma_start(out=evt[:], in_=ev_v[:, bass.ds(i, 1), :])
                nc.sync.dma_start(out=polt[:], in_=pol_v[:, bass.ds(i, 1), :])
                ev3 = evt[:].rearrange("p (n f) -> p n f", f=4)
                ts4 = ev3[:, :, 0]
                y4 = ev3[:, :, 1]
                x4 = ev3[:, :, 2]
                pol0 = polt[:].rearrange("p (n f) -> p n f", f=2)[:, :, 0]

                u4 = wpool.tile([PBLK, NBLK], fp32, tag="u4")
                nc.vector.tensor_scalar(out=u4[:], in0=ts4, scalar1=1.0,
                                        scalar2=-1.0, op0=Alu.subtract, op1=Alu.mult)
                sgn4 = wpool.tile([PBLK, NBLK], bf16, tag="sgn4")
                nc.vector.tensor_scalar(out=sgn4[:], in0=pol0, scalar1=2.0,
                                        scalar2=1.0, op0=Alu.mult, op1=Alu.subtract)
                # y rows via strided DMA: yrow4[n, p] = y of event (c,p,n)
                yrow4 = wpool.tile([4, PBLK], fp32, tag="yrow4")
                nc.sync.dma_start(out=yrow4[:], in_=evy_v[:, bass.ds(i, 1), :, 1])
                nc.vector.tensor_copy(out=yb16[0:4, :PBLK], in_=yrow4[:])

                # dy[y, e] = y_e - y  (4 matmuls into p_dy column blocks)
                for j in range(NBLK):
                    nc.tensor.matmul(
                        out=p_dy[:, j * PBLK:(j + 1) * PBLK],
                        lhsT=dylhs[j][:],
                        rhs=yb16[:, :PBLK],
                        start=True, stop=True)

                onehotY = wpool.tile([128, CHUNK], bf16, tag="ohY")
                nc.vector.tensor_scalar(out=onehotY[:], in0=p_dy[:], scalar1=0.0,
                                        scalar2=None, op0=Alu.is_equal)

                # flow row select: T1[e, fy|fx] per block
                for j in range(NBLK):
                    nc.tensor.matmul(
                        out=p_t1[:, j * 256:(j + 1) * 256],
                        lhsT=onehotY[:, j * PBLK:(j + 1) * PBLK],
                        rhs=flowcat[:],
                        start=True, stop=True)

                onehotX = wpool.tile([PBLK, NBLK * 128], fp32, tag="ohX")
                iota_rep = iota32[:PBLK, :].unsqueeze(1).broadcast_to(
                    [PBLK, NBLK, 128])
                x4_bc = x4.unsqueeze(2).broadcast_to([PBLK, NBLK, 128])
                nc.vector.tensor_tensor(
                    out=onehotX[:].rearrange("p (n f) -> p n f", f=128),
                    in0=iota_rep, in1=x4_bc, op=Alu.is_equal)

                # select: prod = T1 * onehotX (repeated over 2 ch), blocked reduce
                prod = wpool.tile([PBLK, NBLK * 256], bf16, tag="prod")
                ohx_rep = onehotX[:].rearrange(
                    "p (n f) -> p n f", f=128).unsqueeze(2).broadcast_to(
                    [PBLK, NBLK, 2, 128])
                nc.vector.tensor_tensor(
                    out=prod[:].rearrange("p (n c f) -> p n c f", c=2, f=128),
                    in0=p_t1[:].rearrange("p (n c f) -> p n c f", c=2, f=128),
                    in1=ohx_rep, op=Alu.mult)
                sel = wpool.tile([PBLK, NBLK * 2], fp32, tag="sel")
                nc.vector.tensor_reduce(
                    out=sel[:],
                    in_=prod[:].rearrange("p (n f) -> p n f", f=128),
                    axis=mybir.AxisListType.X, op=Alu.add)
                sel3 = sel[:].rearrange("p (n f) -> p n f", f=2)
                fy4 = sel3[:, :, 0]
                fx4 = sel3[:, :, 1]

                # warped coords + clamp
                wy4 = wpool.tile([PBLK, NBLK], fp32, tag="wy4")
                wx4 = wpool.tile([PBLK, NBLK], fp32, tag="wx4")
                nc.vector.tensor_tensor(out=wy4[:], in0=u4[:], in1=fy4, op=Alu.mult)
                nc.vector.tensor_tensor(out=wy4[:], in0=wy4[:], in1=y4, op=Alu.add)
                nc.vector.tensor_scalar(out=wy4[:], in0=wy4[:], scalar1=130.0,
                                        scalar2=-2.0, op0=Alu.min, op1=Alu.max)
                nc.vector.tensor_tensor(out=wx4[:], in0=u4[:], in1=fx4, op=Alu.mult)
                nc.vector.tensor_tensor(out=wx4[:], in0=wx4[:], in1=x4, op=Alu.add)
                nc.vector.tensor_scalar(out=wx4[:], in0=wx4[:], scalar1=130.0,
                                        scalar2=-2.0, op0=Alu.min, op1=Alu.max)

                # hats: t = w - iota ; m = |t| ; hat = -(min(m,1) - 1)
                def hat_from(w4, tag):
                    t = wpool.tile([PBLK, NBLK * 128], fp32, tag=tag + "_t")
                    w_bc = w4.unsqueeze(2).broadcast_to([PBLK, NBLK, 128])
                    nc.vector.scalar_tensor_tensor(
                        out=t[:].rearrange("p (n f) -> p n f", f=128),
                        in0=w_bc, scalar=0.0, in1=iota_rep,
                        op0=Alu.add, op1=Alu.subtract)
                    tneg = wpool.tile([PBLK, NBLK * 128], fp32, tag=tag + "_tn")
                    nc.vector.tensor_scalar_mul(out=tneg[:], in0=t[:], scalar1=-1.0)
                    m = wpool.tile([PBLK, NBLK * 128], bf16, tag=tag + "_m")
                    nc.vector.tensor_tensor(out=m[:], in0=t[:], in1=tneg[:],
                                            op=Alu.max)
                    # negated hat: min(|t|,1) - 1 = -relu(1-|t|)
                    h = wpool.tile([PBLK, NBLK * 128], bf16, tag=tag + "_h")
                    nc.vector.tensor_scalar(out=h[:], in0=m[:], scalar1=1.0,
                                            scalar2=1.0, op0=Alu.min,
                                            op1=Alu.subtract)
                    return h

                hatY = hat_from(wy4, "hy")
                hatX = hat_from(wx4, "hx")
                hatYs = wpool.tile([PBLK, NBLK * 128], bf16, tag="hys")
                sgn_bc = sgn4[:].unsqueeze(2).broadcast_to([PBLK, NBLK, 128])
                nc.vector.tensor_tensor(
                    out=hatYs[:].rearrange("p (n f) -> p n f", f=128),
                    in0=hatY[:].rearrange("p (n f) -> p n f", f=128),
                    in1=sgn_bc, op=Alu.mult)

                for j in range(NBLK):
                    sl = slice(j * 128, (j + 1) * 128)
                    nc.tensor.matmul(out=p_gs[:], lhsT=hatY[:, sl],
                                     rhs=hatX[:, sl], start=False, stop=False)
                    nc.tensor.matmul(out=p_gd[:], lhsT=hatYs[:, sl],
                                     rhs=hatX[:, sl], start=False, stop=False)

            if use_hw_loop:
                for _ in range(passes):
                    tc.For_i_unrolled(0, nchunks, 1, body, max_unroll=unroll)
            else:
                for i in range(nchunks):
                    body(i)

            # ---------------- finalize ----------------
            res = cpool.tile([128, 128 * 4], fp32)
            gd_sb = cpool.tile([128, 128], fp32)
            nc.vector.tensor_copy(out=gd_sb[:], in_=p_gd[:])
            nc.vector.tensor_tensor(out=res[:, 0:128], in0=p_gs[:], in1=gd_sb[:],
                                    op=Alu.add)
            nc.vector.tensor_scalar_mul(out=res[:, 0:128], in0=res[:, 0:128],
                                        scalar1=0.5)
            nc.vector.tensor_tensor(out=res[:, 128:256], in0=p_gs[:], in1=gd_sb[:],
                                    op=Alu.subtract)
            nc.vector.tensor_scalar_mul(out=res[:, 128:256], in0=res[:, 128:256],
                                        scalar1=0.5)
            maskt = cpool.tile([128, 128], fp32)
            nc.sync.dma_start(out=maskt[:], in_=emask[:, :])
            inv = 1.0 / (1.0 + 1e-9)
            nc.vector.scalar_tensor_tensor(out=res[:, 256:384],
                                           in0=flow32[:, 128:256], scalar=inv,
                                           in1=maskt[:], op0=Alu.mult, op1=Alu.mult)
            nc.vector.scalar_tensor_tensor(out=res[:, 384:512],
                                           in0=flow32[:, 0:128], scalar=inv,
                                           in1=maskt[:], op0=Alu.mult, op1=Alu.mult)
            for ch in range(4):
                nc.sync.dma_start(out=out[ch], in_=res[:, ch * 128:(ch + 1) * 128])

    nc.compile()
    return nc


def _run(nc, flow, event_list, pol_mask, event_mask):
    """flow [B,2,H,W], event_list [B,N,4], pol [B,N,2], emask [B,1,H,W]."""
    from concourse.bass_utils import run_bass_kernel_spmd

    Bb, Nn = event_list.shape[0], event_list.shape[1]
    half = Nn // 2
    in_maps = []
    for c in range(NCORES):
        b, h = c // 2, c % 2
        sl = slice(h * half, (h + 1) * half)
        in_maps.append({
            "ev": np.ascontiguousarray(event_list[b, sl, :], np.float32),
            "pol": np.ascontiguousarray(pol_mask[b, sl, :], np.float32),
            "flow": np.ascontiguousarray(flow[b], np.float32),
            "emask": np.ascontiguousarray(event_mask[b, 0], np.float32),
        })
    res = run_bass_kernel_spmd(nc, in_maps, list(range(NCORES)))
    out = np.zeros((Bb, 4, H, W), np.float32)
    for c in range(NCORES):
        b = c // 2
        r = res.results[c]["out"]
        out[b, 0:2] += r[0:2]
        if c % 2 == 0:
            out[b, 2:4] = r[2:4]
    return out


def kernel(flow, event_list, pol_mask, event_mask):
    flow = np.asarray(flow, np.float32)
    event_list = np.asarray(event_list, np.float32)
    pol_mask = np.asarray(pol_mask, np.float32)
    event_mask = np.asarray(event_mask, np.float32)
    nchunks = event_list.shape[0] * event_list.shape[1] // NCORES // CHUNK
    key = ("nc", nchunks)
    if key not in _COMPILED:
        _COMPILED[key] = _build(nchunks)
    return _run(_COMPILED[key], flow, event_list, pol_mask, event_mask)



# revision 2
# speedup vs baseline: 1.3851x; 1.3851x over previous
"""V2 Trainium2 kernel for the IWE problem.

Per-core pipeline (events polarity-sorted + padded by host, field3 := 1.0):
  tile = 2048 events = 16 groups of 128 (group = one free column slot across
  all 128 partitions; partition p's row is a contiguous event range in HBM).

  1. PE-transpose the [128, 64] event tile -> field rows (ts,y,x,1) per group.
  2. dy/dx matmuls (lhsT = const [ones/-iota selectors]) -> psum[bin, e] =
     coord - bin; one-hot via is_eq on DVE (y) / Pool (x).
  3. Gather: MM(lhsT=flowpack[bin,x] fp32 packed bf16 (fy|fx), rhs=oYT) ->
     G[x, e]; PT = G * oXT; per-group column-reduce MM with ones -> packed
     flow per event (bit-exact through the matmul).
  4. Unpack fy/fx via bitwise and/shift; wyx_n = -(y + u*f), clamped.
  5. Hats on ACT: m = Abs(iota + bias=wyx_n) per group; h = min(m,1)-1
     (negated hat) on DVE (y) / Pool (x).
  6. Scatter: per group MM(lhsT=hY, rhs=hX) accumulating into G_pos or G_neg
     psum half-bank (polarity = group position, no sign multiply).
"""
import numpy as np

H, W = 128, 128
NCORES = 8
GPT = 16                    # groups per tile
TILE_COLS = GPT * 4         # fp32 cols per tile in the event slab
GPS = 256                   # groups per DMA slab
_COMPILED = {}


def _build(g_pos, g_neg, use_hw_loop=False, passes=1):
    import concourse.bass as bass
    import concourse.bacc as bacc
    import concourse.mybir as mybir
    from concourse.tile import TileContext

    fp32 = mybir.dt.float32
    bf16 = mybir.dt.bfloat16
    int32 = mybir.dt.int32
    Alu = mybir.AluOpType
    Act = mybir.ActivationFunctionType

    G = g_pos + g_neg                      # total groups
    assert g_pos % GPT == 0 and g_neg % GPT == 0
    T_pos, T_neg = g_pos // GPT, g_neg // GPT

    nc = bacc.Bacc("TRN2", target_bir_lowering=False, debug=False,
                   num_devices=NCORES)

    ev = nc.dram_tensor("ev", [128, G * 4], fp32, kind="ExternalInput").ap()
    flowpack = nc.dram_tensor("flowpack", [128, 128], fp32,
                              kind="ExternalInput").ap()
    flow = nc.dram_tensor("flow", [2, H, W], fp32, kind="ExternalInput").ap()
    emask = nc.dram_tensor("emask", [H, W], fp32, kind="ExternalInput").ap()
    c32y_in = nc.dram_tensor("c32y", [64, 1024], fp32, kind="ExternalInput").ap()
    c32x_in = nc.dram_tensor("c32x", [64, 1024], fp32, kind="ExternalInput").ap()
    ident_in = nc.dram_tensor("ident", [128, 128], fp32,
                              kind="ExternalInput").ap()
    out = nc.dram_tensor("out", [4, H, W], fp32, kind="ExternalOutput").ap()

    with TileContext(nc) as tc:
        with tc.tile_pool(name="const", bufs=1) as cpool, \
             tc.tile_pool(name="slab", bufs=2) as spool, \
             tc.tile_pool(name="work", bufs=4) as wpool, \
             tc.tile_pool(name="pgrid", bufs=1, space="PSUM") as gpool, \
             tc.tile_pool(name="ppool1", bufs=1, space="PSUM") as ppool1, \
             tc.tile_pool(name="ppool2", bufs=2, space="PSUM") as ppool2:

            # ---------------- constants ----------------
            c32f = cpool.tile([64, 2048], fp32)
            nc.sync.dma_start(out=c32f[:, 0:1024], in_=c32y_in)
            nc.sync.dma_start(out=c32f[:, 1024:2048], in_=c32x_in)
            c32b = cpool.tile([64, 2048], bf16)
            nc.vector.tensor_copy(out=c32b[:], in_=c32f[:])

            def _sel(base, g):
                hb = 0 if g < 8 else 32
                return c32b[hb:hb + 32, base + (g % 8) * 128:
                            base + (g % 8 + 1) * 128]
            ident = cpool.tile([128, 128], fp32)
            nc.sync.dma_start(out=ident[:], in_=ident_in)
            iotai = cpool.tile([128, 128], int32)
            nc.gpsimd.iota(iotai[:], pattern=[[1, 128]], base=0,
                           channel_multiplier=0)
            iota_b = cpool.tile([128, 128], bf16)
            nc.vector.tensor_copy(out=iota_b[:], in_=iotai[:])
            iotaq = cpool.tile([128, 512], fp32)
            for r in range(4):
                nc.vector.tensor_copy(out=iotaq[:, r * 128:(r + 1) * 128],
                                      in_=iotai[:])
            ones_col = cpool.tile([128, 1], fp32)
            nc.vector.memset(ones_col[:], 1.0)
            zrow = cpool.tile([128, 128], bf16)
            nc.vector.memset(zrow[:], 0.0)
            flowp = cpool.tile([128, 128], fp32)
            nc.sync.dma_start(out=flowp[:], in_=flowpack)
            flow32 = cpool.tile([128, 256], fp32)
            nc.sync.dma_start(out=flow32[:, 0:128], in_=flow[0])
            nc.sync.dma_start(out=flow32[:, 128:256], in_=flow[1])
            maskt = cpool.tile([128, 128], fp32)
            nc.sync.dma_start(out=maskt[:], in_=emask[:, :])

            # persistent grids psum (separate banks = separate zero regions)
            grid_pos = gpool.tile([128, 128], fp32, tag="gpos")
            grid_neg = gpool.tile([128, 128], fp32, tag="gneg")
            # open both accumulation groups with zero matmuls
            nc.tensor.matmul(out=grid_pos[:], lhsT=zrow[:], rhs=zrow[:],
                             start=True, stop=False)
            nc.tensor.matmul(out=grid_neg[:], lhsT=zrow[:], rhs=zrow[:],
                             start=True, stop=False)

            def tile_body(slab_t, tt, pos_region):
                """Process tile tt (0..15) of the current slab."""
                ev64 = slab_t[:, tt * TILE_COLS:(tt + 1) * TILE_COLS]
                ev3 = ev64.rearrange("p (g f) -> p g f", f=4)

                # --- transpose -> field rows ---
                p_tr = ppool1.tile([64, 128], fp32, tag="tr")
                nc.tensor.transpose(out=p_tr[:], in_=ev64, identity=ident[:])
                trT = wpool.tile([64, 128], bf16, tag="trT")
                nc.scalar.copy(out=trT[:], in_=p_tr[:])

                # --- per-tile smalls ---
                uneg = wpool.tile([128, GPT], fp32, tag="uneg")
                nc.vector.tensor_scalar(out=uneg[:], in0=ev3[:, :, 0],
                                        scalar1=1.0, scalar2=None,
                                        op0=Alu.subtract)

                # --- select pipeline per quad ---
                p_sel = ppool1.tile([128, GPT], fp32, tag="sel")
                oyts, oxts = [], []
                for q in range(4):
                    p_dy = ppool2.tile([128, 512], fp32, tag="dy")
                    p_dx = ppool1.tile([128, 512], fp32, tag="dx")
                    for j in range(4):
                        g = 4 * q + j
                        w = trT[0:32, :] if g < 8 else trT[32:64, :]
                        nc.tensor.matmul(
                            out=p_dy[:, j * 128:(j + 1) * 128],
                            lhsT=_sel(0, g), rhs=w, start=True, stop=True)
                    for j in range(4):
                        g = 4 * q + j
                        w = trT[0:32, :] if g < 8 else trT[32:64, :]
                        nc.tensor.matmul(
                            out=p_dx[:, j * 128:(j + 1) * 128],
                            lhsT=_sel(1024, g), rhs=w, start=True, stop=True)
                    oYT = wpool.tile([128, 512], fp32, tag=f"oyt{q % 2}")
                    nc.vector.tensor_scalar(out=oYT[:], in0=p_dy[:],
                                            scalar1=0.0, scalar2=None,
                                            op0=Alu.is_equal)
                    oXT = wpool.tile([128, 512], fp32, tag=f"oxt{q % 2}")
                    nc.vector.tensor_scalar(out=oXT[:], in0=p_dx[:],
                                            scalar1=0.0, scalar2=None,
                                            op0=Alu.is_equal)
                    p_g = ppool1.tile([128, 512], fp32, tag="g")
                    nc.tensor.matmul(out=p_g[:], lhsT=flowp[:], rhs=oYT[:],
                                     start=True, stop=True)
                    PT = wpool.tile([128, 512], fp32, tag=f"pt{q % 2}")
                    nc.vector.tensor_tensor(out=PT[:], in0=p_g[:], in1=oXT[:],
                                            op=Alu.mult)
                    for j in range(4):
                        g = 4 * q + j
                        nc.tensor.matmul(
                            out=p_sel[:, g:g + 1],
                            lhsT=PT[:, j * 128:(j + 1) * 128],
                            rhs=ones_col[:], start=True, stop=True)

                # --- unpack packed flow, warped coords ---
                f32t = wpool.tile([128, 2 * GPT], fp32, tag="f32t")
                f32i = f32t[:].bitcast(int32).rearrange("p (g c) -> p g c", c=2)
                seli = p_sel[:].bitcast(int32)
                nc.vector.tensor_scalar(out=f32i[:, :, 0], in0=seli,
                                        scalar1=-65536, scalar2=None,
                                        op0=Alu.bitwise_and)
                nc.vector.tensor_scalar(out=f32i[:, :, 1], in0=seli,
                                        scalar1=16, scalar2=None,
                                        op0=Alu.logical_shift_left)
                wyxn = wpool.tile([128, 2 * GPT], fp32, tag="wyxn")
                u2 = uneg[:].unsqueeze(2).broadcast_to([128, GPT, 2])
                nc.vector.tensor_tensor(
                    out=wyxn[:].rearrange("p (g c) -> p g c", c=2),
                    in0=f32t[:].rearrange("p (g c) -> p g c", c=2),
                    in1=u2, op=Alu.mult)
                nc.vector.tensor_tensor(
                    out=wyxn[:].rearrange("p (g c) -> p g c", c=2),
                    in0=wyxn[:].rearrange("p (g c) -> p g c", c=2),
                    in1=ev3[:, :, 1:3], op=Alu.subtract)
                nc.vector.tensor_scalar(out=wyxn[:], in0=wyxn[:],
                                        scalar1=-130.0, scalar2=2.0,
                                        op0=Alu.max, op1=Alu.min)

                # --- hats + scatter per quad ---
                for q in range(4):
                    m_y = wpool.tile([128, 512], bf16, tag=f"my{q % 2}")
                    for j in range(4):
                        g = 4 * q + j
                        nc.scalar.activation(
                            out=m_y[:, j * 128:(j + 1) * 128], in_=iota_b[:],
                            func=Act.Abs, bias=wyxn[:, 2 * g:2 * g + 1],
                            scale=1.0)
                    h_y = wpool.tile([128, 512], bf16, tag=f"hy{q % 2}")
                    nc.gpsimd.tensor_scalar(out=h_y[:], in0=m_y[:],
                                            scalar1=1.0, scalar2=1.0,
                                            op0=Alu.min, op1=Alu.subtract)
                    # x-axis: t = iota + (-wx); |t| clamp on Pool; -hat
                    t_x = wpool.tile([128, 512], fp32, tag=f"tx{q % 2}")
                    wxb = wyxn[:].rearrange("p (g c) -> p g c", c=2)[
                        :, 4 * q:4 * q + 4, 1:2].broadcast_to([128, 4, 128])
                    nc.gpsimd.tensor_tensor(
                        out=t_x[:].rearrange("p (g f) -> p g f", f=128),
                        in0=iotaq[:].rearrange("p (g f) -> p g f", f=128),
                        in1=wxb, op=Alu.add)
                    hm_x = wpool.tile([128, 512], bf16, tag=f"hmx{q % 2}")
                    nc.scalar.activation(out=hm_x[:], in_=t_x[:], func=Act.Abs)
                    h_x = wpool.tile([128, 512], bf16, tag=f"hx{q % 2}")
                    nc.gpsimd.tensor_scalar(out=h_x[:], in0=hm_x[:],
                                            scalar1=1.0, scalar2=1.0,
                                            op0=Alu.min, op1=Alu.subtract)
                    gtile = grid_pos if pos_region else grid_neg
                    for j in range(4):
                        nc.tensor.matmul(
                            out=gtile[:],
                            lhsT=h_y[:, j * 128:(j + 1) * 128],
                            rhs=h_x[:, j * 128:(j + 1) * 128],
                            start=False, stop=False)

            gps = min(GPS, g_pos, g_neg)
            n_slab_pos = g_pos // gps
            n_slab_neg = g_neg // gps
            slab_cols = gps * 4

            def slab_body(i, pos_region, base_slabs=0):
                slab_t = spool.tile([128, slab_cols], fp32, tag="slab")
                if isinstance(i, int):
                    off = (base_slabs + i) * slab_cols
                    nc.sync.dma_start(out=slab_t[:], in_=ev[:, off:off + slab_cols])
                else:
                    nc.sync.dma_start(
                        out=slab_t[:],
                        in_=ev[:, bass.ds(base_slabs * slab_cols + i * slab_cols,
                                          slab_cols)])
                for tt in range(gps // GPT):
                    tile_body(slab_t, tt, pos_region)

            for _ in range(passes):
                if use_hw_loop:
                    tc.For_i_unrolled(0, n_slab_pos, 1,
                                      lambda i: slab_body(i, True), max_unroll=1)
                    tc.For_i_unrolled(0, n_slab_neg, 1,
                                      lambda i: slab_body(i, False,
                                                          base_slabs=n_slab_pos),
                                      max_unroll=1)
                else:
                    for i in range(n_slab_pos):
                        slab_body(i, True)
                    for i in range(n_slab_neg):
                        slab_body(i, False, base_slabs=n_slab_pos)

            # close accumulation groups with zero matmuls (stop=True)
            nc.tensor.matmul(out=grid_pos[:], lhsT=zrow[:], rhs=zrow[:],
                             start=False, stop=True)
            nc.tensor.matmul(out=grid_neg[:], lhsT=zrow[:], rhs=zrow[:],
                             start=False, stop=True)

            # ---------------- finalize ----------------
            res = cpool.tile([128, 512], fp32)
            nc.vector.tensor_copy(out=res[:, 0:128], in_=grid_pos[:])
            nc.vector.tensor_copy(out=res[:, 128:256], in_=grid_neg[:])
            inv = 1.0 / (1.0 + 1e-9)
            nc.vector.scalar_tensor_tensor(out=res[:, 256:384],
                                           in0=flow32[:, 0:128], scalar=inv,
                                           in1=maskt[:], op0=Alu.mult,
                                           op1=Alu.mult)
            nc.vector.scalar_tensor_tensor(out=res[:, 384:512],
                                           in0=flow32[:, 128:256], scalar=inv,
                                           in1=maskt[:], op0=Alu.mult,
                                           op1=Alu.mult)
            for ch in range(4):
                nc.sync.dma_start(out=out[ch],
                                  in_=res[:, ch * 128:(ch + 1) * 128])

    nc.compile()
    return nc


def _pack_flow(flow_b):
    """flow_b [2, H, W] -> [H, W] fp32 packed (fy high bf16 | fx low bf16)."""
    fy = np.ascontiguousarray(flow_b[1], np.float32)
    fx = np.ascontiguousarray(flow_b[0], np.float32)
    hy = (fy.view(np.uint32) >> 16).astype(np.uint32)
    hx = (fx.view(np.uint32) >> 16).astype(np.uint32)
    return ((hy << 16) | hx).view(np.float32)


def _consts():
    io = np.arange(128, dtype=np.float32)
    c32y = np.zeros((32, 8, 128), np.float32)
    c32x = np.zeros((32, 8, 128), np.float32)
    for go in range(8):
        c32y[4 * go + 1, go, :] = 1.0
        c32y[4 * go + 3, go, :] = -io
        c32x[4 * go + 2, go, :] = 1.0
        c32x[4 * go + 3, go, :] = -io
    ident = np.eye(128, dtype=np.float32)
    c32y = c32y.reshape(32, 1024)
    c32x = c32x.reshape(32, 1024)
    c32y = np.concatenate([c32y, c32y], axis=0)
    c32x = np.concatenate([c32x, c32x], axis=0)
    return c32y, c32x, ident


def _layout_events(ev_core, g_pos, g_neg):
    """Sort by polarity, pad, arrange [128, (g_pos+g_neg)*4] fp32."""
    p = ev_core[:, 3]
    pos = ev_core[p >= 0.5]
    neg = ev_core[p < 0.5]
    npos_cap, nneg_cap = g_pos * 128, g_neg * 128
    assert len(pos) <= npos_cap and len(neg) <= nneg_cap, \
        (len(pos), len(neg), npos_cap, nneg_cap)
    dead = np.array([0.0, -500.0, -500.0, 1.0], np.float32)

    def region(evs, cap, gcount):
        arr = np.empty((cap, 4), np.float32)
        arr[:len(evs)] = evs
        arr[len(evs):] = dead
        arr[:, 3] = 1.0                       # field3 := ones row source
        return arr.reshape(128, gcount, 4)

    pos_l = region(pos, npos_cap, g_pos)
    neg_l = region(neg, nneg_cap, g_neg)
    full = np.concatenate([pos_l, neg_l], axis=1)       # [128, G, 4]
    return np.ascontiguousarray(full.reshape(128, -1), np.float32)


def _run(nc, flow, event_list, pol_mask, event_mask, g_pos, g_neg):
    from concourse.bass_utils import run_bass_kernel_spmd

    Bb, Nn = event_list.shape[0], event_list.shape[1]
    half = Nn // 2
    c32y, c32x, ident = _consts()
    in_maps = []
    for c in range(NCORES):
        b, h = c // 2, c % 2
        sl = slice(h * half, (h + 1) * half)
        in_maps.append({
            "ev": _layout_events(np.asarray(event_list[b, sl, :], np.float32),
                                 g_pos, g_neg),
            "flowpack": _pack_flow(flow[b]),
            "flow": np.ascontiguousarray(flow[b], np.float32),
            "emask": np.ascontiguousarray(event_mask[b, 0], np.float32),
            "c32y": c32y, "c32x": c32x, "ident": ident,
        })
    res = run_bass_kernel_spmd(nc, in_maps, list(range(NCORES)))
    out = np.zeros((Bb, 4, H, W), np.float32)
    for c in range(NCORES):
        b = c // 2
        r = res.results[c]["out"]
        out[b, 0:2] += r[0:2]
        if c % 2 == 0:
            out[b, 2:4] = r[2:4]
    return out


G_POS_FULL = 2048
G_NEG_FULL = 2048


def kernel(flow, event_list, pol_mask, event_mask):
    flow = np.asarray(flow, np.float32)
    event_list = np.asarray(event_list, np.float32)
    pol_mask = np.asarray(pol_mask, np.float32)
    event_mask = np.asarray(event_mask, np.float32)
    key = ("nc", G_POS_FULL, G_NEG_FULL)
    if key not in _COMPILED:
        _COMPILED[key] = _build(G_POS_FULL, G_NEG_FULL, use_hw_loop=True)
    return _run(_COMPILED[key], flow, event_list, pol_mask, event_mask,
                G_POS_FULL, G_NEG_FULL)


# revision 3
# speedup vs baseline: 1.8489x; 1.3349x over previous
"""V2 Trainium2 kernel for the IWE problem.

Per-core pipeline (events polarity-sorted + padded by host, field3 := 1.0):
  tile = 2048 events = 16 groups of 128 (group = one free column slot across
  all 128 partitions; partition p's row is a contiguous event range in HBM).

  1. PE-transpose the [128, 64] event tile -> field rows (ts,y,x,1) per group.
  2. dy/dx matmuls (lhsT = const [ones/-iota selectors]) -> psum[bin, e] =
     coord - bin; one-hot via is_eq on DVE (y) / Pool (x).
  3. Gather: MM(lhsT=flowpack[bin,x] fp32 packed bf16 (fy|fx), rhs=oYT) ->
     G[x, e]; PT = G * oXT; per-group column-reduce MM with ones -> packed
     flow per event (bit-exact through the matmul).
  4. Unpack fy/fx via bitwise and/shift; wyx_n = -(y + u*f), clamped.
  5. Hats on ACT: m = Abs(iota + bias=wyx_n) per group; h = min(m,1)-1
     (negated hat) on DVE (y) / Pool (x).
  6. Scatter: per group MM(lhsT=hY, rhs=hX) accumulating into G_pos or G_neg
     psum half-bank (polarity = group position, no sign multiply).
"""
import numpy as np

H, W = 128, 128
NCORES = 8
GPT = 16                    # groups per tile
TILE_COLS = GPT * 4         # fp32 cols per tile in the event slab
GPS = 256                   # groups per DMA slab
_COMPILED = {}


def _build(g_pos, g_neg, use_hw_loop=False, passes=1):
    import concourse.bass as bass
    import concourse.bacc as bacc
    import concourse.mybir as mybir
    from concourse.tile import TileContext

    fp32 = mybir.dt.float32
    bf16 = mybir.dt.bfloat16
    int32 = mybir.dt.int32
    Alu = mybir.AluOpType
    Act = mybir.ActivationFunctionType

    G = g_pos + g_neg                      # total groups
    assert g_pos % GPT == 0 and g_neg % GPT == 0
    T_pos, T_neg = g_pos // GPT, g_neg // GPT

    nc = bacc.Bacc("TRN2", target_bir_lowering=False, debug=False,
                   num_devices=NCORES)

    ev = nc.dram_tensor("ev", [128, G * 4], fp32, kind="ExternalInput").ap()
    flowpack = nc.dram_tensor("flowpack", [128, 128], fp32,
                              kind="ExternalInput").ap()
    flow = nc.dram_tensor("flow", [2, H, W], fp32, kind="ExternalInput").ap()
    emask = nc.dram_tensor("emask", [H, W], fp32, kind="ExternalInput").ap()
    c32y_in = nc.dram_tensor("c32y", [64, 1024], fp32, kind="ExternalInput").ap()
    c32x_in = nc.dram_tensor("c32x", [64, 1024], fp32, kind="ExternalInput").ap()
    ident_in = nc.dram_tensor("ident", [128, 128], fp32,
                              kind="ExternalInput").ap()
    out = nc.dram_tensor("out", [4, H, W], fp32, kind="ExternalOutput").ap()

    with TileContext(nc) as tc:
        with tc.tile_pool(name="const", bufs=1) as cpool, \
             tc.tile_pool(name="slab", bufs=2) as spool, \
             tc.tile_pool(name="work", bufs=2) as wpool, \
             tc.tile_pool(name="pgrid", bufs=1, space="PSUM") as gpool, \
             tc.tile_pool(name="ppool1", bufs=1, space="PSUM") as ppool1, \
             tc.tile_pool(name="ppool2", bufs=2, space="PSUM") as ppool2:

            # ---------------- constants ----------------
            c32f = cpool.tile([64, 2048], fp32)
            nc.sync.dma_start(out=c32f[:, 0:1024], in_=c32y_in)
            nc.sync.dma_start(out=c32f[:, 1024:2048], in_=c32x_in)
            c32b = cpool.tile([64, 2048], bf16)
            nc.vector.tensor_copy(out=c32b[:], in_=c32f[:])

            def _sel(base, g):
                hb = 0 if g < 8 else 32
                return c32b[hb:hb + 32, base + (g % 8) * 128:
                            base + (g % 8 + 1) * 128]
            ident = cpool.tile([128, 128], fp32)
            nc.sync.dma_start(out=ident[:], in_=ident_in)
            iotai = cpool.tile([128, 128], int32)
            nc.gpsimd.iota(iotai[:], pattern=[[1, 128]], base=0,
                           channel_multiplier=0)
            iota_b = cpool.tile([128, 128], bf16)
            nc.vector.tensor_copy(out=iota_b[:], in_=iotai[:])
            iotaq16 = cpool.tile([128, 2048], fp32)
            for r in range(16):
                nc.vector.tensor_copy(out=iotaq16[:, r * 128:(r + 1) * 128],
                                      in_=iotai[:])
            ones_col = cpool.tile([128, 1], fp32)
            nc.vector.memset(ones_col[:], 1.0)
            zrow = cpool.tile([128, 128], bf16)
            nc.vector.memset(zrow[:], 0.0)
            flowp = cpool.tile([128, 128], fp32)
            nc.sync.dma_start(out=flowp[:], in_=flowpack)
            flow32 = cpool.tile([128, 256], fp32)
            nc.sync.dma_start(out=flow32[:, 0:128], in_=flow[0])
            nc.sync.dma_start(out=flow32[:, 128:256], in_=flow[1])
            maskt = cpool.tile([128, 128], fp32)
            nc.sync.dma_start(out=maskt[:], in_=emask[:, :])

            # persistent grids psum (separate banks = separate zero regions)
            grid_pos = gpool.tile([128, 128], fp32, tag="gpos")
            grid_neg = gpool.tile([128, 128], fp32, tag="gneg")
            # open both accumulation groups with zero matmuls
            nc.tensor.matmul(out=grid_pos[:], lhsT=zrow[:], rhs=zrow[:],
                             start=True, stop=False)
            nc.tensor.matmul(out=grid_neg[:], lhsT=zrow[:], rhs=zrow[:],
                             start=True, stop=False)

            def tile_body(slab_t, tt, pos_region):
                """Process tile tt (0..15) of the current slab."""
                ev64 = slab_t[:, tt * TILE_COLS:(tt + 1) * TILE_COLS]
                ev3 = ev64.rearrange("p (g f) -> p g f", f=4)

                # --- transpose -> field rows ---
                p_tr = ppool1.tile([64, 128], fp32, tag="tr")
                nc.tensor.transpose(out=p_tr[:], in_=ev64, identity=ident[:])
                trT = wpool.tile([64, 128], bf16, tag="trT")
                nc.scalar.copy(out=trT[:], in_=p_tr[:])

                # --- per-tile smalls ---
                uneg = wpool.tile([128, GPT], fp32, tag="uneg")
                nc.vector.tensor_scalar(out=uneg[:], in0=ev3[:, :, 0],
                                        scalar1=1.0, scalar2=None,
                                        op0=Alu.subtract)

                # --- select pipeline per quad ---
                p_sel = ppool1.tile([128, GPT], fp32, tag="sel")
                oyts, oxts = [], []
                for q in range(4):
                    p_dy = ppool2.tile([128, 512], fp32, tag="dy")
                    p_dx = ppool1.tile([128, 512], fp32, tag="dx")
                    for j in range(4):
                        g = 4 * q + j
                        w = trT[0:32, :] if g < 8 else trT[32:64, :]
                        nc.tensor.matmul(
                            out=p_dy[:, j * 128:(j + 1) * 128],
                            lhsT=_sel(0, g), rhs=w, start=True, stop=True)
                    for j in range(4):
                        g = 4 * q + j
                        w = trT[0:32, :] if g < 8 else trT[32:64, :]
                        nc.tensor.matmul(
                            out=p_dx[:, j * 128:(j + 1) * 128],
                            lhsT=_sel(1024, g), rhs=w, start=True, stop=True)
                    oYT = wpool.tile([128, 512], fp32, tag=f"oyt{q % 2}")
                    nc.vector.tensor_scalar(out=oYT[:], in0=p_dy[:],
                                            scalar1=0.0, scalar2=None,
                                            op0=Alu.is_equal)
                    oXT = wpool.tile([128, 512], fp32, tag=f"oxt{q % 2}")
                    nc.vector.tensor_scalar(out=oXT[:], in0=p_dx[:],
                                            scalar1=0.0, scalar2=None,
                                            op0=Alu.is_equal)
                    p_g = ppool1.tile([128, 512], fp32, tag="g")
                    nc.tensor.matmul(out=p_g[:], lhsT=flowp[:], rhs=oYT[:],
                                     start=True, stop=True)
                    PT = wpool.tile([128, 512], fp32, tag=f"pt{q % 2}")
                    nc.vector.tensor_tensor(out=PT[:], in0=p_g[:], in1=oXT[:],
                                            op=Alu.mult)
                    for j in range(4):
                        g = 4 * q + j
                        nc.tensor.matmul(
                            out=p_sel[:, g:g + 1],
                            lhsT=PT[:, j * 128:(j + 1) * 128],
                            rhs=ones_col[:], start=True, stop=True)

                # --- unpack packed flow, warped coords ---
                f32t = wpool.tile([128, 2 * GPT], fp32, tag="f32t")
                f32i = f32t[:].bitcast(int32).rearrange("p (g c) -> p g c", c=2)
                seli = p_sel[:].bitcast(int32)
                nc.vector.tensor_scalar(out=f32i[:, :, 0], in0=seli,
                                        scalar1=-65536, scalar2=None,
                                        op0=Alu.bitwise_and)
                nc.vector.tensor_scalar(out=f32i[:, :, 1], in0=seli,
                                        scalar1=16, scalar2=None,
                                        op0=Alu.logical_shift_left)
                wyxn = wpool.tile([128, 2 * GPT], fp32, tag="wyxn")
                u2 = uneg[:].unsqueeze(2).broadcast_to([128, GPT, 2])
                nc.vector.tensor_tensor(
                    out=wyxn[:].rearrange("p (g c) -> p g c", c=2),
                    in0=f32t[:].rearrange("p (g c) -> p g c", c=2),
                    in1=u2, op=Alu.mult)
                nc.vector.tensor_tensor(
                    out=wyxn[:].rearrange("p (g c) -> p g c", c=2),
                    in0=wyxn[:].rearrange("p (g c) -> p g c", c=2),
                    in1=ev3[:, :, 1:3], op=Alu.subtract)
                nc.vector.tensor_scalar(out=wyxn[:], in0=wyxn[:],
                                        scalar1=-130.0, scalar2=2.0,
                                        op0=Alu.max, op1=Alu.min)

                # --- hats: full-tile-wide ops ---
                wyv = wyxn[:].rearrange("p (g c) -> p g c", c=2)
                t_y = wpool.tile([128, 2048], fp32, tag="ty")
                nc.gpsimd.tensor_tensor(
                    out=t_y[:].rearrange("p (g f) -> p g f", f=128),
                    in0=iotaq16[:].rearrange("p (g f) -> p g f", f=128),
                    in1=wyv[:, :, 0:1].broadcast_to([128, GPT, 128]),
                    op=Alu.add)
                t_x = wpool.tile([128, 2048], fp32, tag="tx")
                nc.gpsimd.tensor_tensor(
                    out=t_x[:].rearrange("p (g f) -> p g f", f=128),
                    in0=iotaq16[:].rearrange("p (g f) -> p g f", f=128),
                    in1=wyv[:, :, 1:2].broadcast_to([128, GPT, 128]),
                    op=Alu.add)
                hm_y = wpool.tile([128, 2048], bf16, tag="hmy")
                nc.scalar.activation(
                    out=hm_y[:].rearrange("p (g f) -> p g f", f=128),
                    in_=t_y[:].rearrange("p (g f) -> p g f", f=128),
                    func=Act.Abs)
                hm_x = wpool.tile([128, 2048], bf16, tag="hmx")
                nc.scalar.activation(
                    out=hm_x[:].rearrange("p (g f) -> p g f", f=128),
                    in_=t_x[:].rearrange("p (g f) -> p g f", f=128),
                    func=Act.Abs)
                h_y = wpool.tile([128, 2048], bf16, tag="hy")
                nc.gpsimd.tensor_scalar(
                    out=h_y[:].rearrange("p (g f) -> p g f", f=128),
                    in0=hm_y[:].rearrange("p (g f) -> p g f", f=128),
                    scalar1=1.0, scalar2=1.0, op0=Alu.min, op1=Alu.subtract)
                h_x = wpool.tile([128, 2048], bf16, tag="hx")
                nc.gpsimd.tensor_scalar(
                    out=h_x[:].rearrange("p (g f) -> p g f", f=128),
                    in0=hm_x[:].rearrange("p (g f) -> p g f", f=128),
                    scalar1=1.0, scalar2=1.0, op0=Alu.min, op1=Alu.subtract)
                gtile = grid_pos if pos_region else grid_neg
                for g in range(GPT):
                    nc.tensor.matmul(
                        out=gtile[:],
                        lhsT=h_y[:, g * 128:(g + 1) * 128],
                        rhs=h_x[:, g * 128:(g + 1) * 128],
                        start=False, stop=False)

            gps = min(GPS, g_pos, g_neg)
            n_slab_pos = g_pos // gps
            n_slab_neg = g_neg // gps
            slab_cols = gps * 4

            def slab_body(i, pos_region, base_slabs=0):
                slab_t = spool.tile([128, slab_cols], fp32, tag="slab")
                if isinstance(i, int):
                    off = (base_slabs + i) * slab_cols
                    nc.sync.dma_start(out=slab_t[:], in_=ev[:, off:off + slab_cols])
                else:
                    nc.sync.dma_start(
                        out=slab_t[:],
                        in_=ev[:, bass.ds(base_slabs * slab_cols + i * slab_cols,
                                          slab_cols)])
                for tt in range(gps // GPT):
                    tile_body(slab_t, tt, pos_region)

            for _ in range(passes):
                if use_hw_loop:
                    tc.For_i_unrolled(0, n_slab_pos, 1,
                                      lambda i: slab_body(i, True), max_unroll=1)
                    tc.For_i_unrolled(0, n_slab_neg, 1,
                                      lambda i: slab_body(i, False,
                                                          base_slabs=n_slab_pos),
                                      max_unroll=1)
                else:
                    for i in range(n_slab_pos):
                        slab_body(i, True)
                    for i in range(n_slab_neg):
                        slab_body(i, False, base_slabs=n_slab_pos)

            # close accumulation groups with zero matmuls (stop=True)
            nc.tensor.matmul(out=grid_pos[:], lhsT=zrow[:], rhs=zrow[:],
                             start=False, stop=True)
            nc.tensor.matmul(out=grid_neg[:], lhsT=zrow[:], rhs=zrow[:],
                             start=False, stop=True)

            # ---------------- finalize ----------------
            res = cpool.tile([128, 512], fp32)
            nc.vector.tensor_copy(out=res[:, 0:128], in_=grid_pos[:])
            nc.vector.tensor_copy(out=res[:, 128:256], in_=grid_neg[:])
            inv = 1.0 / (1.0 + 1e-9)
            nc.vector.scalar_tensor_tensor(out=res[:, 256:384],
                                           in0=flow32[:, 0:128], scalar=inv,
                                           in1=maskt[:], op0=Alu.mult,
                                           op1=Alu.mult)
            nc.vector.scalar_tensor_tensor(out=res[:, 384:512],
                                           in0=flow32[:, 128:256], scalar=inv,
                                           in1=maskt[:], op0=Alu.mult,
                                           op1=Alu.mult)
            for ch in range(4):
                nc.sync.dma_start(out=out[ch],
                                  in_=res[:, ch * 128:(ch + 1) * 128])

    nc.compile()
    return nc


def _pack_flow(flow_b):
    """flow_b [2, H, W] -> [H, W] fp32 packed (fy high bf16 | fx low bf16)."""
    fy = np.ascontiguousarray(flow_b[1], np.float32)
    fx = np.ascontiguousarray(flow_b[0], np.float32)
    hy = (fy.view(np.uint32) >> 16).astype(np.uint32)
    hx = (fx.view(np.uint32) >> 16).astype(np.uint32)
    return ((hy << 16) | hx).view(np.float32)


def _consts():
    io = np.arange(128, dtype=np.float32)
    c32y = np.zeros((32, 8, 128), np.float32)
    c32x = np.zeros((32, 8, 128), np.float32)
    for go in range(8):
        c32y[4 * go + 1, go, :] = 1.0
        c32y[4 * go + 3, go, :] = -io
        c32x[4 * go + 2, go, :] = 1.0
        c32x[4 * go + 3, go, :] = -io
    ident = np.eye(128, dtype=np.float32)
    c32y = c32y.reshape(32, 1024)
    c32x = c32x.reshape(32, 1024)
    c32y = np.concatenate([c32y, c32y], axis=0)
    c32x = np.concatenate([c32x, c32x], axis=0)
    return c32y, c32x, ident


def _layout_events(ev_core, g_pos, g_neg):
    """Sort by polarity, pad, arrange [128, (g_pos+g_neg)*4] fp32."""
    p = ev_core[:, 3]
    pos = ev_core[p >= 0.5]
    neg = ev_core[p < 0.5]
    npos_cap, nneg_cap = g_pos * 128, g_neg * 128
    assert len(pos) <= npos_cap and len(neg) <= nneg_cap, \
        (len(pos), len(neg), npos_cap, nneg_cap)
    dead = np.array([0.0, -500.0, -500.0, 1.0], np.float32)

    def region(evs, cap, gcount):
        arr = np.empty((cap, 4), np.float32)
        arr[:len(evs)] = evs
        arr[len(evs):] = dead
        arr[:, 3] = 1.0                       # field3 := ones row source
        return arr.reshape(128, gcount, 4)

    pos_l = region(pos, npos_cap, g_pos)
    neg_l = region(neg, nneg_cap, g_neg)
    full = np.concatenate([pos_l, neg_l], axis=1)       # [128, G, 4]
    return np.ascontiguousarray(full.reshape(128, -1), np.float32)


def _run(nc, flow, event_list, pol_mask, event_mask, g_pos, g_neg):
    from concourse.bass_utils import run_bass_kernel_spmd

    Bb, Nn = event_list.shape[0], event_list.shape[1]
    half = Nn // 2
    c32y, c32x, ident = _consts()
    in_maps = []
    for c in range(NCORES):
        b, h = c // 2, c % 2
        sl = slice(h * half, (h + 1) * half)
        in_maps.append({
            "ev": _layout_events(np.asarray(event_list[b, sl, :], np.float32),
                                 g_pos, g_neg),
            "flowpack": _pack_flow(flow[b]),
            "flow": np.ascontiguousarray(flow[b], np.float32),
            "emask": np.ascontiguousarray(event_mask[b, 0], np.float32),
            "c32y": c32y, "c32x": c32x, "ident": ident,
        })
    res = run_bass_kernel_spmd(nc, in_maps, list(range(NCORES)))
    out = np.zeros((Bb, 4, H, W), np.float32)
    for c in range(NCORES):
        b = c // 2
        r = res.results[c]["out"]
        out[b, 0:2] += r[0:2]
        if c % 2 == 0:
            out[b, 2:4] = r[2:4]
    return out


G_POS_FULL = 2048
G_NEG_FULL = 2048


def kernel(flow, event_list, pol_mask, event_mask):
    flow = np.asarray(flow, np.float32)
    event_list = np.asarray(event_list, np.float32)
    pol_mask = np.asarray(pol_mask, np.float32)
    event_mask = np.asarray(event_mask, np.float32)
    key = ("nc", G_POS_FULL, G_NEG_FULL)
    if key not in _COMPILED:
        _COMPILED[key] = _build(G_POS_FULL, G_NEG_FULL, use_hw_loop=True)
    return _run(_COMPILED[key], flow, event_list, pol_mask, event_mask,
                G_POS_FULL, G_NEG_FULL)


# revision 4
# speedup vs baseline: 2.1101x; 1.1413x over previous
"""V2 Trainium2 kernel for the IWE problem.

Per-core pipeline (events polarity-sorted + padded by host, field3 := 1.0):
  tile = 2048 events = 16 groups of 128 (group = one free column slot across
  all 128 partitions; partition p's row is a contiguous event range in HBM).

  1. PE-transpose the [128, 64] event tile -> field rows (ts,y,x,1) per group.
  2. dy/dx matmuls (lhsT = const [ones/-iota selectors]) -> psum[bin, e] =
     coord - bin; one-hot via is_eq on DVE (y) / Pool (x).
  3. Gather: MM(lhsT=flowpack[bin,x] fp32 packed bf16 (fy|fx), rhs=oYT) ->
     G[x, e]; PT = G * oXT; per-group column-reduce MM with ones -> packed
     flow per event (bit-exact through the matmul).
  4. Unpack fy/fx via bitwise and/shift; wyx_n = -(y + u*f), clamped.
  5. Hats on ACT: m = Abs(iota + bias=wyx_n) per group; h = min(m,1)-1
     (negated hat) on DVE (y) / Pool (x).
  6. Scatter: per group MM(lhsT=hY, rhs=hX) accumulating into G_pos or G_neg
     psum half-bank (polarity = group position, no sign multiply).
"""
import numpy as np

H, W = 128, 128
NCORES = 8
GPT = 16                    # groups per tile
TILE_COLS = GPT * 4         # fp32 cols per tile in the event slab
GPS = 256                   # groups per DMA slab
_COMPILED = {}


def _build(g_pos, g_neg, use_hw_loop=False, passes=1):
    import concourse.bass as bass
    import concourse.bacc as bacc
    import concourse.mybir as mybir
    from concourse.tile import TileContext

    fp32 = mybir.dt.float32
    bf16 = mybir.dt.bfloat16
    int32 = mybir.dt.int32
    Alu = mybir.AluOpType
    Act = mybir.ActivationFunctionType

    G = g_pos + g_neg                      # total groups
    assert g_pos % GPT == 0 and g_neg % GPT == 0
    T_pos, T_neg = g_pos // GPT, g_neg // GPT

    nc = bacc.Bacc("TRN2", target_bir_lowering=False, debug=False,
                   num_devices=NCORES)

    ev = nc.dram_tensor("ev", [128, G * 4], fp32, kind="ExternalInput").ap()
    flowpack = nc.dram_tensor("flowpack", [128, 128], fp32,
                              kind="ExternalInput").ap()
    flow = nc.dram_tensor("flow", [2, H, W], fp32, kind="ExternalInput").ap()
    emask = nc.dram_tensor("emask", [H, W], fp32, kind="ExternalInput").ap()
    yxcat_in = nc.dram_tensor("yxcat", [1, G * 256], fp32,
                              kind="ExternalInput").ap()
    out = nc.dram_tensor("out", [4, H, W], fp32, kind="ExternalOutput").ap()

    with TileContext(nc) as tc:
        with tc.tile_pool(name="const", bufs=1) as cpool, \
             tc.tile_pool(name="slab", bufs=2) as spool, \
             tc.tile_pool(name="work", bufs=2) as wpool, \
             tc.tile_pool(name="pgrid", bufs=1, space="PSUM") as gpool, \
             tc.tile_pool(name="ppool1", bufs=1, space="PSUM") as ppool1, \
             tc.tile_pool(name="ppool2", bufs=2, space="PSUM") as ppool2:

            # ---------------- constants ----------------
            ones_row = cpool.tile([1, 128], fp32)
            nc.vector.memset(ones_row[:], 1.0)
            pidi = cpool.tile([128, 1], int32)
            nc.gpsimd.iota(pidi[:], pattern=[[1, 1]], base=0,
                           channel_multiplier=1)
            pidf = cpool.tile([128, 1], fp32)
            nc.vector.tensor_copy(out=pidf[:], in_=pidi[:])
            iotai = cpool.tile([128, 128], int32)
            nc.gpsimd.iota(iotai[:], pattern=[[1, 128]], base=0,
                           channel_multiplier=0)
            iota_b = cpool.tile([128, 128], bf16)
            nc.vector.tensor_copy(out=iota_b[:], in_=iotai[:])
            iotaq16 = cpool.tile([128, 2048], fp32)
            for r in range(16):
                nc.vector.tensor_copy(out=iotaq16[:, r * 128:(r + 1) * 128],
                                      in_=iotai[:])
            ones_col = cpool.tile([128, 1], fp32)
            nc.vector.memset(ones_col[:], 1.0)
            zrow = cpool.tile([128, 128], bf16)
            nc.vector.memset(zrow[:], 0.0)
            flowp = cpool.tile([128, 128], fp32)
            nc.sync.dma_start(out=flowp[:], in_=flowpack)
            flow32 = cpool.tile([128, 256], fp32)
            nc.sync.dma_start(out=flow32[:, 0:128], in_=flow[0])
            nc.sync.dma_start(out=flow32[:, 128:256], in_=flow[1])
            maskt = cpool.tile([128, 128], fp32)
            nc.sync.dma_start(out=maskt[:], in_=emask[:, :])

            # persistent grids psum (separate banks = separate zero regions)
            grid_pos = gpool.tile([128, 128], fp32, tag="gpos")
            grid_neg = gpool.tile([128, 128], fp32, tag="gneg")
            # open both accumulation groups with zero matmuls
            nc.tensor.matmul(out=grid_pos[:], lhsT=zrow[:], rhs=zrow[:],
                             start=True, stop=False)
            nc.tensor.matmul(out=grid_neg[:], lhsT=zrow[:], rhs=zrow[:],
                             start=True, stop=False)

            def tile_body(slab_t, yx_src, tt, pos_region):
                """Process tile tt (0..15) of the current slab."""
                ev64 = slab_t[:, tt * TILE_COLS:(tt + 1) * TILE_COLS]
                ev3 = ev64.rearrange("p (g f) -> p g f", f=4)
                yx_t = wpool.tile([1, 4096], fp32, tag="yxt")
                nc.sync.dma_start(out=yx_t[:], in_=yx_src)

                # --- per-tile smalls ---
                uneg = wpool.tile([128, GPT], fp32, tag="uneg")
                nc.vector.tensor_scalar(out=uneg[:], in0=ev3[:, :, 0],
                                        scalar1=1.0, scalar2=None,
                                        op0=Alu.subtract)

                # --- select pipeline per quad ---
                p_sel = ppool1.tile([128, GPT], fp32, tag="sel")
                oyts, oxts = [], []
                for q in range(4):
                    p_dy = ppool2.tile([128, 512], fp32, tag="dy")
                    p_dx = ppool1.tile([128, 512], fp32, tag="dx")
                    nc.tensor.matmul(out=p_dy[:], lhsT=ones_row[:],
                                     rhs=yx_t[:, q * 512:(q + 1) * 512],
                                     start=True, stop=True)
                    nc.tensor.matmul(out=p_dx[:], lhsT=ones_row[:],
                                     rhs=yx_t[:, 2048 + q * 512:
                                              2048 + (q + 1) * 512],
                                     start=True, stop=True)
                    oYT = wpool.tile([128, 512], fp32, tag=f"oyt{q % 2}")
                    nc.vector.tensor_tensor(
                        out=oYT[:], in0=p_dy[:],
                        in1=pidf[:].broadcast_to([128, 512]), op=Alu.is_equal)
                    oXT = wpool.tile([128, 512], fp32, tag=f"oxt{q % 2}")
                    nc.vector.tensor_tensor(
                        out=oXT[:], in0=p_dx[:],
                        in1=pidf[:].broadcast_to([128, 512]), op=Alu.is_equal)
                    p_g = ppool1.tile([128, 512], fp32, tag="g")
                    nc.tensor.matmul(out=p_g[:], lhsT=flowp[:], rhs=oYT[:],
                                     start=True, stop=True)
                    PT = wpool.tile([128, 512], fp32, tag=f"pt{q % 2}")
                    nc.vector.tensor_tensor(out=PT[:], in0=p_g[:], in1=oXT[:],
                                            op=Alu.mult)
                    for j in range(4):
                        g = 4 * q + j
                        nc.tensor.matmul(
                            out=p_sel[:, g:g + 1],
                            lhsT=PT[:, j * 128:(j + 1) * 128],
                            rhs=ones_col[:], start=True, stop=True)

                # --- unpack packed flow, warped coords ---
                f32t = wpool.tile([128, 2 * GPT], fp32, tag="f32t")
                f32i = f32t[:].bitcast(int32).rearrange("p (g c) -> p g c", c=2)
                seli = p_sel[:].bitcast(int32)
                nc.vector.tensor_scalar(out=f32i[:, :, 0], in0=seli,
                                        scalar1=-65536, scalar2=None,
                                        op0=Alu.bitwise_and)
                nc.vector.tensor_scalar(out=f32i[:, :, 1], in0=seli,
                                        scalar1=16, scalar2=None,
                                        op0=Alu.logical_shift_left)
                wyxn = wpool.tile([128, 2 * GPT], fp32, tag="wyxn")
                u2 = uneg[:].unsqueeze(2).broadcast_to([128, GPT, 2])
                nc.vector.tensor_tensor(
                    out=wyxn[:].rearrange("p (g c) -> p g c", c=2),
                    in0=f32t[:].rearrange("p (g c) -> p g c", c=2),
                    in1=u2, op=Alu.mult)
                nc.vector.tensor_tensor(
                    out=wyxn[:].rearrange("p (g c) -> p g c", c=2),
                    in0=wyxn[:].rearrange("p (g c) -> p g c", c=2),
                    in1=ev3[:, :, 1:3], op=Alu.subtract)
                nc.vector.tensor_scalar(out=wyxn[:], in0=wyxn[:],
                                        scalar1=-130.0, scalar2=2.0,
                                        op0=Alu.max, op1=Alu.min)

                # --- hats: full-tile-wide ops ---
                wyv = wyxn[:].rearrange("p (g c) -> p g c", c=2)
                t_y = wpool.tile([128, 2048], fp32, tag="ty")
                nc.gpsimd.tensor_tensor(
                    out=t_y[:].rearrange("p (g f) -> p g f", f=128),
                    in0=iotaq16[:].rearrange("p (g f) -> p g f", f=128),
                    in1=wyv[:, :, 0:1].broadcast_to([128, GPT, 128]),
                    op=Alu.add)
                t_x = wpool.tile([128, 2048], fp32, tag="tx")
                nc.gpsimd.tensor_tensor(
                    out=t_x[:].rearrange("p (g f) -> p g f", f=128),
                    in0=iotaq16[:].rearrange("p (g f) -> p g f", f=128),
                    in1=wyv[:, :, 1:2].broadcast_to([128, GPT, 128]),
                    op=Alu.add)
                hm_y = wpool.tile([128, 2048], bf16, tag="hmy")
                nc.scalar.activation(
                    out=hm_y[:].rearrange("p (g f) -> p g f", f=128),
                    in_=t_y[:].rearrange("p (g f) -> p g f", f=128),
                    func=Act.Abs)
                hm_x = wpool.tile([128, 2048], bf16, tag="hmx")
                nc.scalar.activation(
                    out=hm_x[:].rearrange("p (g f) -> p g f", f=128),
                    in_=t_x[:].rearrange("p (g f) -> p g f", f=128),
                    func=Act.Abs)
                h_y = wpool.tile([128, 2048], bf16, tag="hy")
                nc.gpsimd.tensor_scalar(
                    out=h_y[:].rearrange("p (g f) -> p g f", f=128),
                    in0=hm_y[:].rearrange("p (g f) -> p g f", f=128),
                    scalar1=1.0, scalar2=1.0, op0=Alu.min, op1=Alu.subtract)
                h_x = wpool.tile([128, 2048], bf16, tag="hx")
                nc.gpsimd.tensor_scalar(
                    out=h_x[:].rearrange("p (g f) -> p g f", f=128),
                    in0=hm_x[:].rearrange("p (g f) -> p g f", f=128),
                    scalar1=1.0, scalar2=1.0, op0=Alu.min, op1=Alu.subtract)
                gtile = grid_pos if pos_region else grid_neg
                for g in range(GPT):
                    nc.tensor.matmul(
                        out=gtile[:],
                        lhsT=h_y[:, g * 128:(g + 1) * 128],
                        rhs=h_x[:, g * 128:(g + 1) * 128],
                        start=False, stop=False)

            gps = min(GPS, g_pos, g_neg)
            n_slab_pos = g_pos // gps
            n_slab_neg = g_neg // gps
            slab_cols = gps * 4

            def slab_body(i, pos_region, base_slabs=0):
                slab_t = spool.tile([128, slab_cols], fp32, tag="slab")
                yxc = gps * 256
                if isinstance(i, int):
                    off = (base_slabs + i) * slab_cols
                    nc.sync.dma_start(out=slab_t[:], in_=ev[:, off:off + slab_cols])
                    yxo = (base_slabs + i) * yxc
                    srcs = [yxcat_in[:, yxo + tt * 4096:yxo + (tt + 1) * 4096]
                            for tt in range(gps // GPT)]
                else:
                    nc.sync.dma_start(
                        out=slab_t[:],
                        in_=ev[:, bass.ds(base_slabs * slab_cols + i * slab_cols,
                                          slab_cols)])
                    srcs = [yxcat_in[:, bass.ds(base_slabs * yxc + i * yxc
                                                + tt * 4096, 4096)]
                            for tt in range(gps // GPT)]
                for tt in range(gps // GPT):
                    tile_body(slab_t, srcs[tt], tt, pos_region)

            for _ in range(passes):
                if use_hw_loop:
                    tc.For_i_unrolled(0, n_slab_pos, 1,
                                      lambda i: slab_body(i, True), max_unroll=1)
                    tc.For_i_unrolled(0, n_slab_neg, 1,
                                      lambda i: slab_body(i, False,
                                                          base_slabs=n_slab_pos),
                                      max_unroll=1)
                else:
                    for i in range(n_slab_pos):
                        slab_body(i, True)
                    for i in range(n_slab_neg):
                        slab_body(i, False, base_slabs=n_slab_pos)

            # close accumulation groups with zero matmuls (stop=True)
            nc.tensor.matmul(out=grid_pos[:], lhsT=zrow[:], rhs=zrow[:],
                             start=False, stop=True)
            nc.tensor.matmul(out=grid_neg[:], lhsT=zrow[:], rhs=zrow[:],
                             start=False, stop=True)

            # ---------------- finalize ----------------
            res = cpool.tile([128, 512], fp32)
            nc.vector.tensor_copy(out=res[:, 0:128], in_=grid_pos[:])
            nc.vector.tensor_copy(out=res[:, 128:256], in_=grid_neg[:])
            inv = 1.0 / (1.0 + 1e-9)
            nc.vector.scalar_tensor_tensor(out=res[:, 256:384],
                                           in0=flow32[:, 0:128], scalar=inv,
                                           in1=maskt[:], op0=Alu.mult,
                                           op1=Alu.mult)
            nc.vector.scalar_tensor_tensor(out=res[:, 384:512],
                                           in0=flow32[:, 128:256], scalar=inv,
                                           in1=maskt[:], op0=Alu.mult,
                                           op1=Alu.mult)
            for ch in range(4):
                nc.sync.dma_start(out=out[ch],
                                  in_=res[:, ch * 128:(ch + 1) * 128])

    nc.compile()
    return nc


def _pack_flow(flow_b):
    """flow_b [2, H, W] -> [H, W] fp32 packed (fy high bf16 | fx low bf16)."""
    fy = np.ascontiguousarray(flow_b[1], np.float32)
    fx = np.ascontiguousarray(flow_b[0], np.float32)
    hy = (fy.view(np.uint32) >> 16).astype(np.uint32)
    hx = (fx.view(np.uint32) >> 16).astype(np.uint32)
    return ((hy << 16) | hx).view(np.float32)


def _build_yxcat(ev_l, g_pos, g_neg):
    G = g_pos + g_neg
    T = G // GPT
    evg = ev_l.reshape(128, G, 4)
    y = evg[:, :, 1].reshape(128, T, GPT).transpose(1, 2, 0)   # [T, g, e]
    x = evg[:, :, 2].reshape(128, T, GPT).transpose(1, 2, 0)
    yx = np.concatenate([y.reshape(T, 1, GPT * 128),
                         x.reshape(T, 1, GPT * 128)], axis=1)  # [T, 2, 2048]
    return np.ascontiguousarray(yx.reshape(1, -1), np.float32)


def _layout_events(ev_core, g_pos, g_neg):
    """Sort by polarity, pad, arrange [128, (g_pos+g_neg)*4] fp32."""
    p = ev_core[:, 3]
    pos = ev_core[p >= 0.5]
    neg = ev_core[p < 0.5]
    npos_cap, nneg_cap = g_pos * 128, g_neg * 128
    assert len(pos) <= npos_cap and len(neg) <= nneg_cap, \
        (len(pos), len(neg), npos_cap, nneg_cap)
    dead = np.array([0.0, -500.0, -500.0, 1.0], np.float32)

    def region(evs, cap, gcount):
        arr = np.empty((cap, 4), np.float32)
        arr[:len(evs)] = evs
        arr[len(evs):] = dead
        arr[:, 3] = 1.0                       # field3 := ones row source
        return arr.reshape(128, gcount, 4)

    pos_l = region(pos, npos_cap, g_pos)
    neg_l = region(neg, nneg_cap, g_neg)
    full = np.concatenate([pos_l, neg_l], axis=1)       # [128, G, 4]
    return np.ascontiguousarray(full.reshape(128, -1), np.float32)


def _run(nc, flow, event_list, pol_mask, event_mask, g_pos, g_neg):
    from concourse.bass_utils import run_bass_kernel_spmd

    Bb, Nn = event_list.shape[0], event_list.shape[1]
    half = Nn // 2

    in_maps = []
    for c in range(NCORES):
        b, h = c // 2, c % 2
        sl = slice(h * half, (h + 1) * half)
        ev_l = _layout_events(np.asarray(event_list[b, sl, :], np.float32),
                              g_pos, g_neg)
        in_maps.append({
            "ev": ev_l,
            "yxcat": _build_yxcat(ev_l, g_pos, g_neg),
            "flowpack": _pack_flow(flow[b]),
            "flow": np.ascontiguousarray(flow[b], np.float32),
            "emask": np.ascontiguousarray(event_mask[b, 0], np.float32),
        })
    res = run_bass_kernel_spmd(nc, in_maps, list(range(NCORES)))
    out = np.zeros((Bb, 4, H, W), np.float32)
    for c in range(NCORES):
        b = c // 2
        r = res.results[c]["out"]
        out[b, 0:2] += r[0:2]
        if c % 2 == 0:
            out[b, 2:4] = r[2:4]
    return out


G_POS_FULL = 2048
G_NEG_FULL = 2048


def kernel(flow, event_list, pol_mask, event_mask):
    flow = np.asarray(flow, np.float32)
    event_list = np.asarray(event_list, np.float32)
    pol_mask = np.asarray(pol_mask, np.float32)
    event_mask = np.asarray(event_mask, np.float32)
    key = ("nc", G_POS_FULL, G_NEG_FULL)
    if key not in _COMPILED:
        _COMPILED[key] = _build(G_POS_FULL, G_NEG_FULL, use_hw_loop=True)
    return _run(_COMPILED[key], flow, event_list, pol_mask, event_mask,
                G_POS_FULL, G_NEG_FULL)
